# revision 50
# baseline (speedup 1.0000x reference)
"""GCN layer (h@W scaled by norm, gather/scatter-sum over edges, norm+bias+relu)
as a distributed Bass kernel on 8 TRN2 NeuronCores.

Strategy:
  out = relu(norm_dst * ((A @ (norm_src*h)) @ W) + bias)   [linearity of matmul]
  - dst nodes sharded 2500/core (padded to 20 blocks of 128 slots).
  - hs table = (norm*h) in fp8e3 (e3m4) replicated to every core's HBM:
    norm_src is FOLDED INTO THE TABLE, so the scatter matrix S holds exact
    small-int edge multiplicities (fp8e3, lossless). 512B gather rows halve
    DMA-ring descriptor time vs 16-bit.
  - Edges bucketed by (core, dst-block) on host, padded per block to a
    schedule that is IDENTICAL across cores (blocks sorted by size so the
    per-rank max is tight) -> one SPMD program for all 8 cores.
  - The DMA rings' descriptor throughput + serial SWDGE generation pace the
    random-row gathers, so everything else stays out of their way: gathers
    are merged in PAIRS of blocks (after two half-split singles that start
    the pipeline fast, and a tiny warmup gather that absorbs the ~10us
    first-SWDGE-use latency), idx tiles load per group ahead of all other
    traffic, W is pre-transposed on host into one contiguous load, S rides
    the scalar HWDGE queue per group, and outputs store in slot-major pairs
    (2KB-per-partition descriptors).
  - Per block: TensorE matmul-accumulate S.T @ G into PSUM = x_block
    [128, 512] -> x scaled by norm_dst via ScalarE activation-scale (fp16
    out) -> PE-transpose -> fp16 projection with W + bias via a rank-1
    ones x fp8-bias matmul in the same PSUM accumulation group -> Relu on
    ScalarE straight from PSUM into the fp16 output buffer. Block j's
    post-agg chain is issued after block j+1's agg matmuls so PE never
    stalls on the scale/transpose round-trip.
"""

import numpy as np
import ml_dtypes

import concourse.bacc as bacc
import concourse.mybir as mybir
import concourse.tile as tile
from concourse._compat import cdiv
from concourse.masks import make_identity

N_CORES = 8
BS = 128  # dst block size == partition count
N_SWDGE_QUEUES = 4
GBUFS = 7  # gather pool buffers (groups); deep so SWDGE generation never
           # stalls on WAR reuse of a buffer whose blocks PE hasn't consumed

F32 = mybir.dt.float32
F16 = mybir.dt.float16
F8 = mybir.dt.float8e3
F8E4 = mybir.dt.float8e4
I16 = mybir.dt.int16
NP_F8 = ml_dtypes.float8_e3m4


def _groups(nblk):
    """Gather groups: two singles to start the pipeline, then pairs."""
    gs = [[0], [1]]
    j = 2
    while j < nblk:
        gs.append(list(range(j, min(j + 2, nblk))))
        j += 2
    return gs


def _prepare(h, weight, bias, norm, src, dst):
    """Host-side sharding/preprocessing. Returns (nc, in_maps, meta)."""
    h = np.asarray(h, dtype=np.float32)
    weight = np.asarray(weight, dtype=np.float32)
    bias = np.asarray(bias, dtype=np.float32).reshape(1, -1)
    norm = np.asarray(norm, dtype=np.float32).reshape(-1)
    src = np.asarray(src).astype(np.int64)
    dst = np.asarray(dst).astype(np.int64)

    n_nodes, d_in = h.shape
    d_out = weight.shape[1]
    assert d_in % BS == 0 and d_out % BS == 0
    assert n_nodes % N_CORES == 0
    npc = n_nodes // N_CORES          # nodes per core
    nblk = cdiv(npc, BS)              # dst blocks per core
    npc_pad = nblk * BS

    hs8 = (norm[:, None] * h).astype(NP_F8)

    # Bucket edges by (core, block); slot within block.
    core_of = dst // npc
    local = dst - core_of * npc
    blk_of = local // BS
    slot_of = (local % BS).astype(np.int64)

    order = np.lexsort((blk_of, core_of))
    e_sorted = order
    cb = core_of[order] * nblk + blk_of[order]
    counts = np.bincount(cb, minlength=N_CORES * nblk).reshape(N_CORES, nblk)

    # Dedup: gather each distinct src row once per (core, block); the
    # host-built S tiles fold edge multiplicity (rows get several nonzeros).
    starts0 = np.zeros(N_CORES * nblk + 1, np.int64)
    np.cumsum(counts.reshape(-1), out=starts0[1:])
    uniq_lists = {}
    inv_lists = {}
    tiles = np.zeros((N_CORES, nblk), np.int64)
    for c in range(N_CORES):
        for j in range(nblk):
            s, e = starts0[c * nblk + j], starts0[c * nblk + j + 1]
            idx = e_sorted[s:e]
            uniq, inv = np.unique(src[idx], return_inverse=True)
            uniq_lists[(c, j)] = (uniq, idx)
            inv_lists[(c, j)] = inv
            tiles[c, j] = -(-max(len(uniq), 1) // BS)

    # Common schedule: sort each core's blocks by tile count desc;
    # schedule rank j gets max over cores of j-th largest.
    perm = np.argsort(-tiles, axis=1, kind="stable")      # [C, nblk]
    sorted_tiles = np.take_along_axis(tiles, perm, axis=1)
    t_sched = np.maximum(sorted_tiles.max(axis=0), 1)     # [nblk]
    # visit the smallest block first so TensorE starts sooner, then
    # largest -> smallest (equal-size pairing measured ~25us slower: the
    # big pairs' generation belongs early, overlapping the PE-light phase)
    visit = np.concatenate(([nblk - 1], np.arange(nblk - 1)))
    t_sched = t_sched[visit]
    perm = perm[:, visit]
    t_total = int(t_sched.sum())
    e_pad = t_total * BS

    src_pack = np.zeros((N_CORES, e_pad), np.int16)
    stab32 = np.zeros((BS, t_total * BS), np.float32)
    stab = np.zeros((N_CORES, BS, t_total * BS), NP_F8)
    perms = []
    for c in range(N_CORES):
        stab32[:] = 0.0
        off = 0  # in gather-stream positions (edges)
        for j in range(nblk):
            b = int(perm[c, j])
            uniq, idx = uniq_lists[(c, b)]
            inv = inv_lists[(c, b)]
            src_pack[c, off:off + len(uniq)] = uniq.astype(np.int16)
            # edge e of this bucket -> stream row (off + inv[e]), col slot;
            # norm_src lives in the hs table, so S counts multiplicity only
            # (exact small ints -> fp8e3 lossless).
            rows = off + inv
            np.add.at(stab32, (rows % BS, (rows // BS) * BS + slot_of[idx]),
                      1.0)
            off += int(t_sched[j]) * BS
        stab[c] = stab32.astype(NP_F8)
        perms.append(perm[c])

    def wrap16(a):  # [e_pad] -> [128, e_pad//16] (16-partition wrap, x8 copies)
        return np.tile(a.reshape(-1, 16).T, (8, 1))

    # norm_dst per core in schedule order [128, nblk]
    ndst = np.zeros((N_CORES, BS, nblk), np.float32)
    for c in range(N_CORES):
        padded = np.zeros(npc_pad, np.float32)
        padded[:npc] = norm[c * npc:(c + 1) * npc]
        blocks = padded.reshape(nblk, BS)
        ndst[c] = blocks[perm[c]].T

    # W pre-arranged on host for one contiguous DMA: [p, k*d_out]
    kin = d_in // BS
    w16 = weight.astype(np.float16).reshape(kin, BS, d_out)     # [k, p, n]
    w16 = np.ascontiguousarray(w16.transpose(1, 0, 2)).reshape(BS, kin * d_out)

    in_maps = []
    for c in range(N_CORES):
        in_maps.append({
            "htab": hs8,
            "wmat": w16,
            "brow": bias.astype(ml_dtypes.float8_e4m3),
            "ndst": ndst[c],
            "gidx": wrap16(src_pack[c]).astype(np.int16),
            "stab": stab[c],
        })

    nc = _build(n_nodes, d_in, d_out, nblk, [int(t) for t in t_sched])

    meta = dict(npc=npc, nblk=nblk, npc_pad=npc_pad, perms=perms,
                n_nodes=n_nodes, d_out=d_out)
    return nc, in_maps, meta


def _build(n_nodes, d_in, d_out, nblk, t_sched):
    """Build the SPMD single-core program (same for all cores)."""
    kin = d_in // BS
    t_total = sum(t_sched)
    e_pad = t_total * BS
    groups = _groups(nblk)

    nc = bacc.Bacc("TRN2", target_bir_lowering=False, debug=False,
                   num_swdge_queues=N_SWDGE_QUEUES)
    htab = nc.dram_tensor("htab", [n_nodes, d_in], F8, kind="ExternalInput")
    wmat = nc.dram_tensor("wmat", [BS, kin * d_out], F16, kind="ExternalInput")
    brow = nc.dram_tensor("brow", [1, d_out], F8E4, kind="ExternalInput")
    ndst = nc.dram_tensor("ndst", [BS, nblk], F32, kind="ExternalInput")
    gidx = nc.dram_tensor("gidx", [128, e_pad // 16], I16, kind="ExternalInput")
    stab = nc.dram_tensor("stab", [BS, t_total * BS], F8, kind="ExternalInput")
    yout = nc.dram_tensor("yout", [BS, nblk * d_out], F16, kind="ExternalOutput")

    with tile.TileContext(nc) as tc:
        with (
            tc.tile_pool(name="const", bufs=1) as cpool,
            tc.tile_pool(name="gather", bufs=GBUFS) as gpool,
            tc.tile_pool(name="sload", bufs=6) as spool,
            tc.tile_pool(name="work", bufs=6) as wpool,
            tc.tile_pool(name="psx", bufs=3, space="PSUM") as psx,
            tc.tile_pool(name="pst", bufs=2, space="PSUM") as pst,
            tc.tile_pool(name="pso", bufs=2, space="PSUM") as pso,
        ):
            # per-group idx tiles, group 0's FIRST: a DMA's completion sem
            # requires every ring to pass its markers, so a tiny first load
            # completes fast instead of trailing the whole startup burst
            idxts = []
            _o = 0
            for n_, g_ in enumerate(groups):
                _t = sum(t_sched[j] for j in g_)
                it = cpool.tile([128, _t * 8], I16, tag=f"idx{n_}",
                                name=f"idx{n_}")
                nc.sync.dma_start(it[:], gidx[:, _o * 8:(_o + _t) * 8])
                idxts.append(it)
                _o += _t
            ws = cpool.tile([128, kin * d_out], F16)
            bs_t = cpool.tile([1, d_out], F8E4)
            ns_t = cpool.tile([BS, nblk], F32)
            # fp8 bias row: exact enough, and an fp8 rhs streams one column
            # per cycle through PE vs two for 16-bit
            ident = cpool.tile([BS, BS], F16)
            ones = cpool.tile([1, BS], F16)

            # tiny warmup gather: absorbs the fixed first-SWDGE-use latency
            # (~10us) while the real idx tiles are still loading
            warm_idx = cpool.tile([128, 8], I16, tag="warmidx")
            nc.gpsimd.memset(warm_idx[:], 0)
            warm_g = cpool.tile([128, 1, d_in], F8, tag="warmg")
            nc.gpsimd.dma_gather(warm_g[:, 0:1, :], htab[:, :], warm_idx[:],
                                 BS, BS, d_in, single_packet=False,
                                 queue_num=3)

            gmax = max(sum(t_sched[j] for j in g) for g in groups)
            t_max = max(t_sched)
            pxs = [None] * nblk  # live px PSUM tiles (one stage of pipelining)
            sts = {}             # block j -> (S tile, column offset)
            opair = [None]       # current output pair tile

            def finish_block(j):
                """Post-aggregation chain for block j (norm_dst, transpose,
                project+bias, relu into the paired output buffer + store)."""
                xs = wpool.tile([BS, d_in], F16, tag="xs")
                # norm_dst scale rides ScalarE's activation scale: keeps DVE
                # off the px->transpose chain and frees the PSUM bank sooner
                nc.scalar.activation(xs[:], pxs[j][:],
                                     mybir.ActivationFunctionType.Copy,
                                     scale=ns_t[:, j:j + 1])
                xT = wpool.tile([128, kin, BS], F16, tag="xT")
                for k in range(kin):
                    tp = pst.tile([BS, BS], F16, tag="tp")
                    nc.tensor.transpose(tp[:], xs[:, k * BS:(k + 1) * BS], ident[:])
                    nc.vector.tensor_copy(xT[:, k, :], tp[:])
                po = pso.tile([BS, d_out], F32, tag="po")
                # bias via rank-1 (ones^T @ brow) matmul opens the group
                nc.tensor.matmul(po[:], ones[:], bs_t[:], start=True, stop=False)
                for k in range(kin):
                    nc.tensor.matmul(po[:], xT[:, k, :],
                                     ws[:, k * d_out:(k + 1) * d_out],
                                     start=False, stop=(k == kin - 1))
                # two blocks share one output tile -> one 2KB-per-partition
                # store for each pair (halves store descriptor count)
                if j % 2 == 0:
                    opair[0] = wpool.tile([BS, 2, d_out], F16, tag="op",
                                          name="op")
                nc.scalar.activation(opair[0][:, j % 2, :], po[:],
                                     mybir.ActivationFunctionType.Relu)
                if j % 2 == 1:
                    nc.sync.dma_start(
                        yout[:, (j - 1) * d_out:(j + 1) * d_out], opair[0][:])
                elif j == nblk - 1:
                    nc.sync.dma_start(
                        yout[:, j * d_out:(j + 1) * d_out], opair[0][:, 0, :])

            off = 0      # edge-tile offset
            done = -1    # last block whose post-agg chain was issued
            for gi, grp in enumerate(groups):
                tg = sum(t_sched[j] for j in grp)
                g = gpool.tile([128, gmax, d_in], F8, tag="g")
                # the very first gather is split in two so the first agg
                # matmuls can start roughly half a generation earlier
                chunks = ([(0, tg // 2), (tg // 2, tg - tg // 2)]
                          if gi == 0 else [(0, tg)])
                for co, cn in chunks:
                    nc.gpsimd.dma_gather(
                        g[:, co:co + cn, :], htab[:, :],
                        idxts[gi][:, co * 8:(co + cn) * 8],
                        cn * BS, cn * BS, d_in, single_packet=False,
                        queue_num=gi % N_SWDGE_QUEUES,
                    )
                # one S load per gather group (bigger descriptors than
                # per-block loads), on the scalar HWDGE queue
                st = spool.tile([BS, gmax * BS], F8, tag="St")
                nc.scalar.dma_start(st[:, 0:tg * BS],
                                    stab[:, off * BS:(off + tg) * BS])
                if gi == 0:
                    # constants built while block-0's gather drains; kept off
                    # GpSimd's critical path ahead of the first gather
                    make_identity(nc, ident[:])
                    nc.gpsimd.memset(ones[:], 1.0)
                elif gi == 1:
                    # W/bias/norm not needed until the first finish_block
                    # (issued later this iteration); loading them here keeps
                    # the startup ring burst small
                    nc.scalar.dma_start(ws[:], wmat[:])
                    nc.scalar.dma_start(bs_t[:], brow[:])
                    nc.scalar.dma_start(ns_t[:], ndst[:])
                goff = 0  # tile offset within this gather group
                for j in grp:
                    tj = t_sched[j]
                    px = psx.tile([BS, d_in], F32, tag="px")
                    for t in range(tj):
                        pos = goff + t
                        nc.tensor.matmul(px[:], st[:, pos * BS:(pos + 1) * BS],
                                         g[:, goff + t, :], start=(t == 0),
                                         stop=(t == tj - 1))
                    pxs[j] = px
                    goff += tj
                    # issue block j-1's post-agg chain AFTER block j's agg
                    # matmuls so PE isn't blocked waiting on DVE's norm scale
                    if j > 0:
                        finish_block(j - 1)
                    done = j - 1
                off += tg
            for j in range(done + 1, nblk):
                finish_block(j)

    nc.compile()
    return nc


def _assemble(results, meta):
    n_nodes, d_out = meta["n_nodes"], meta["d_out"]
    npc, nblk = meta["npc"], meta["nblk"]
    out = np.empty((n_nodes, d_out), np.float32)
    for c in range(N_CORES):
        res = np.asarray(results[c]["yout"]).astype(np.float32)
        res = res.reshape(BS, nblk, d_out)
        for j in range(nblk):
            b = int(meta["perms"][c][j])
            lo = b * BS
            n = min(BS, npc - lo)
            if n > 0:
                out[c * npc + lo: c * npc + lo + n] = res[:n, j, :]
    return out


def kernel(h, weight, bias, norm, src, dst):
    from concourse.bass_utils import run_bass_kernel_spmd
    nc, in_maps, meta = _prepare(h, weight, bias, norm, src, dst)
    r = run_bass_kernel_spmd(nc, in_maps, list(range(N_CORES)))
    return _assemble(r.results, meta)


# revision 51
# speedup vs baseline: 1.0603x; 1.0603x over previous
"""GCN layer (h@W scaled by norm, gather/scatter-sum over edges, norm+bias+relu)
as a distributed Bass kernel on 8 TRN2 NeuronCores.

Strategy:
  out = relu(norm_dst * ((A @ (norm_src*h)) @ W) + bias)   [linearity of matmul]
  - dst nodes sharded 2500/core (padded to 20 blocks of 128 slots).
  - hs table = (norm*h) in fp8e3 (e3m4) replicated to every core's HBM:
    norm_src is FOLDED INTO THE TABLE, so the scatter matrix S holds exact
    small-int edge multiplicities (fp8e3, lossless). 512B gather rows halve
    DMA-ring descriptor time vs 16-bit.
  - Edges bucketed by (core, dst-block) on host, padded per block to a
    schedule that is IDENTICAL across cores (blocks sorted by size so the
    per-rank max is tight) -> one SPMD program for all 8 cores.
  - The DMA rings' descriptor throughput + serial SWDGE generation pace the
    random-row gathers, so everything else stays out of their way: gathers
    are merged in PAIRS of blocks (after two half-split singles that start
    the pipeline fast, and a tiny warmup gather that absorbs the ~10us
    first-SWDGE-use latency), idx tiles load per group ahead of all other
    traffic, W is pre-transposed on host into one contiguous load, S rides
    the scalar HWDGE queue per group, and outputs store in slot-major pairs
    (2KB-per-partition descriptors).
  - Per block: TensorE matmul-accumulate S.T @ G into PSUM = x_block
    [128, 512] -> x scaled by norm_dst via ScalarE activation-scale (fp16
    out) -> PE-transpose -> fp16 projection with W + bias via a rank-1
    ones x fp8-bias matmul in the same PSUM accumulation group -> Relu on
    ScalarE straight from PSUM into the fp16 output buffer. Block j's
    post-agg chain is issued after block j+1's agg matmuls so PE never
    stalls on the scale/transpose round-trip.
"""

import numpy as np
import ml_dtypes

import concourse.bacc as bacc
import concourse.mybir as mybir
import concourse.tile as tile
from concourse._compat import cdiv
from concourse.masks import make_identity

N_CORES = 8
BS = 128  # dst block size == partition count
N_SWDGE_QUEUES = 4
GBUFS = 7  # gather pool buffers (groups); deep so SWDGE generation never
           # stalls on WAR reuse of a buffer whose blocks PE hasn't consumed

F32 = mybir.dt.float32
F16 = mybir.dt.float16
F8 = mybir.dt.float8e3
F8E4 = mybir.dt.float8e4
I16 = mybir.dt.int16
NP_F8 = ml_dtypes.float8_e3m4


def _groups(nblk):
    """Gather groups: two singles to start the pipeline, then pairs."""
    gs = [[0], [1]]
    j = 2
    while j < nblk:
        gs.append(list(range(j, min(j + 2, nblk))))
        j += 2
    return gs


def _prepare(h, weight, bias, norm, src, dst):
    """Host-side sharding/preprocessing. Returns (nc, in_maps, meta)."""
    h = np.asarray(h, dtype=np.float32)
    weight = np.asarray(weight, dtype=np.float32)
    bias = np.asarray(bias, dtype=np.float32).reshape(1, -1)
    norm = np.asarray(norm, dtype=np.float32).reshape(-1)
    src = np.asarray(src).astype(np.int64)
    dst = np.asarray(dst).astype(np.int64)

    n_nodes, d_in = h.shape
    d_out = weight.shape[1]
    assert d_in % BS == 0 and d_out % BS == 0
    assert n_nodes % N_CORES == 0
    npc = n_nodes // N_CORES          # nodes per core
    nblk = cdiv(npc, BS)              # dst blocks per core
    npc_pad = nblk * BS

    hs8 = (norm[:, None] * h).astype(NP_F8)

    # Bucket edges by (core, block); slot within block.
    core_of = dst // npc
    local = dst - core_of * npc
    blk_of = local // BS
    slot_of = (local % BS).astype(np.int64)

    order = np.lexsort((blk_of, core_of))
    e_sorted = order
    cb = core_of[order] * nblk + blk_of[order]
    counts = np.bincount(cb, minlength=N_CORES * nblk).reshape(N_CORES, nblk)

    # Dedup: gather each distinct src row once per (core, block); the
    # host-built S tiles fold edge multiplicity (rows get several nonzeros).
    starts0 = np.zeros(N_CORES * nblk + 1, np.int64)
    np.cumsum(counts.reshape(-1), out=starts0[1:])
    uniq_lists = {}
    inv_lists = {}
    tiles = np.zeros((N_CORES, nblk), np.int64)
    for c in range(N_CORES):
        for j in range(nblk):
            s, e = starts0[c * nblk + j], starts0[c * nblk + j + 1]
            idx = e_sorted[s:e]
            uniq, inv = np.unique(src[idx], return_inverse=True)
            uniq_lists[(c, j)] = (uniq, idx)
            inv_lists[(c, j)] = inv
            tiles[c, j] = -(-max(len(uniq), 1) // BS)

    # Common schedule: sort each core's blocks by tile count desc;
    # schedule rank j gets max over cores of j-th largest.
    perm = np.argsort(-tiles, axis=1, kind="stable")      # [C, nblk]
    sorted_tiles = np.take_along_axis(tiles, perm, axis=1)
    t_sched = np.maximum(sorted_tiles.max(axis=0), 1)     # [nblk]
    # visit the smallest block first so TensorE starts sooner, then
    # largest -> smallest (equal-size pairing measured ~25us slower: the
    # big pairs' generation belongs early, overlapping the PE-light phase)
    visit = np.concatenate(([nblk - 1], np.arange(nblk - 1)))
    t_sched = t_sched[visit]
    perm = perm[:, visit]
    t_total = int(t_sched.sum())
    e_pad = t_total * BS

    src_pack = np.zeros((N_CORES, e_pad), np.int16)
    stab32 = np.zeros((BS, t_total * BS), np.float32)
    stab = np.zeros((N_CORES, BS, t_total * BS), NP_F8)
    perms = []
    for c in range(N_CORES):
        stab32[:] = 0.0
        off = 0  # in gather-stream positions (edges)
        for j in range(nblk):
            b = int(perm[c, j])
            uniq, idx = uniq_lists[(c, b)]
            inv = inv_lists[(c, b)]
            src_pack[c, off:off + len(uniq)] = uniq.astype(np.int16)
            # edge e of this bucket -> stream row (off + inv[e]), col slot;
            # norm_src lives in the hs table, so S counts multiplicity only
            # (exact small ints -> fp8e3 lossless).
            rows = off + inv
            np.add.at(stab32, (rows % BS, (rows // BS) * BS + slot_of[idx]),
                      1.0)
            off += int(t_sched[j]) * BS
        stab[c] = stab32.astype(NP_F8)
        perms.append(perm[c])

    def wrap16(a):  # [e_pad] -> [128, e_pad//16] (16-partition wrap, x8 copies)
        return np.tile(a.reshape(-1, 16).T, (8, 1))

    # norm_dst per core in schedule order [128, nblk]
    ndst = np.zeros((N_CORES, BS, nblk), np.float32)
    for c in range(N_CORES):
        padded = np.zeros(npc_pad, np.float32)
        padded[:npc] = norm[c * npc:(c + 1) * npc]
        blocks = padded.reshape(nblk, BS)
        ndst[c] = blocks[perm[c]].T

    # W pre-arranged on host for one contiguous DMA: [p, k*d_out]
    kin = d_in // BS
    w16 = weight.astype(np.float16).reshape(kin, BS, d_out)     # [k, p, n]
    w16 = np.ascontiguousarray(w16.transpose(1, 0, 2)).reshape(BS, kin * d_out)

    in_maps = []
    for c in range(N_CORES):
        in_maps.append({
            "htab": hs8,
            "wmat": w16,
            "brow": bias.astype(ml_dtypes.float8_e4m3),
            "ndst": ndst[c],
            "gidx": wrap16(src_pack[c]).astype(np.int16),
            "stab": stab[c],
        })

    nc = _build(n_nodes, d_in, d_out, nblk, [int(t) for t in t_sched])

    meta = dict(npc=npc, nblk=nblk, npc_pad=npc_pad, perms=perms,
                n_nodes=n_nodes, d_out=d_out)
    return nc, in_maps, meta


def _build(n_nodes, d_in, d_out, nblk, t_sched):
    """Build the SPMD single-core program (same for all cores)."""
    kin = d_in // BS
    t_total = sum(t_sched)
    e_pad = t_total * BS
    groups = _groups(nblk)

    nc = bacc.Bacc("TRN2", target_bir_lowering=False, debug=False,
                   num_swdge_queues=N_SWDGE_QUEUES)
    htab = nc.dram_tensor("htab", [n_nodes, d_in], F8, kind="ExternalInput")
    wmat = nc.dram_tensor("wmat", [BS, kin * d_out], F16, kind="ExternalInput")
    brow = nc.dram_tensor("brow", [1, d_out], F8E4, kind="ExternalInput")
    ndst = nc.dram_tensor("ndst", [BS, nblk], F32, kind="ExternalInput")
    gidx = nc.dram_tensor("gidx", [128, e_pad // 16], I16, kind="ExternalInput")
    stab = nc.dram_tensor("stab", [BS, t_total * BS], F8, kind="ExternalInput")
    yout = nc.dram_tensor("yout", [BS, nblk * d_out], F16, kind="ExternalOutput")

    with tile.TileContext(nc) as tc:
        with (
            tc.tile_pool(name="const", bufs=1) as cpool,
            tc.tile_pool(name="gather", bufs=GBUFS) as gpool,
            tc.tile_pool(name="sload", bufs=6) as spool,
            tc.tile_pool(name="work", bufs=6) as wpool,
            tc.tile_pool(name="psx", bufs=3, space="PSUM") as psx,
            tc.tile_pool(name="pst", bufs=2, space="PSUM") as pst,
            tc.tile_pool(name="pso", bufs=2, space="PSUM") as pso,
        ):
            # per-group idx tiles, group 0's FIRST: a DMA's completion sem
            # requires every ring to pass its markers, so a tiny first load
            # completes fast instead of trailing the whole startup burst
            idxts = []
            _o = 0
            for n_, g_ in enumerate(groups):
                _t = sum(t_sched[j] for j in g_)
                it = cpool.tile([128, _t * 8], I16, tag=f"idx{n_}",
                                name=f"idx{n_}")
                nc.sync.dma_start(it[:], gidx[:, _o * 8:(_o + _t) * 8])
                idxts.append(it)
                _o += _t
            ws = cpool.tile([128, kin * d_out], F16)
            bs_t = cpool.tile([1, d_out], F8E4)
            ns_t = cpool.tile([BS, nblk], F32)
            # fp8 bias row: exact enough, and an fp8 rhs streams one column
            # per cycle through PE vs two for 16-bit
            ident = cpool.tile([BS, BS], F16)
            ones = cpool.tile([1, BS], F16)

            # tiny warmup gather: absorbs the fixed first-SWDGE-use latency
            # (~10us) while the real idx tiles are still loading
            warm_idx = cpool.tile([128, 8], I16, tag="warmidx")
            nc.gpsimd.memset(warm_idx[:], 0)
            warm_g = cpool.tile([128, 1, d_in], F8, tag="warmg")
            nc.gpsimd.dma_gather(warm_g[:, 0:1, :], htab[:, :], warm_idx[:],
                                 BS, BS, d_in, single_packet=False,
                                 queue_num=3)

            gmax = max(sum(t_sched[j] for j in g) for g in groups)
            t_max = max(t_sched)
            pxs = [None] * nblk  # live px PSUM tiles (one stage of pipelining)
            sts = {}             # block j -> (S tile, column offset)
            opair = [None]       # current output pair tile

            def finish_block(j):
                """Post-aggregation chain for block j (norm_dst, transpose,
                project+bias, relu into the paired output buffer + store)."""
                xs = wpool.tile([BS, d_in], F16, tag="xs")
                # norm_dst scale rides ScalarE's activation scale: keeps DVE
                # off the px->transpose chain and frees the PSUM bank sooner
                nc.scalar.activation(xs[:], pxs[j][:],
                                     mybir.ActivationFunctionType.Copy,
                                     scale=ns_t[:, j:j + 1])
                xT = wpool.tile([128, kin, BS], F16, tag="xT")
                for k in range(kin):
                    tp = pst.tile([BS, BS], F16, tag="tp")
                    nc.tensor.transpose(tp[:], xs[:, k * BS:(k + 1) * BS], ident[:])
                    nc.vector.tensor_copy(xT[:, k, :], tp[:])
                po = pso.tile([BS, d_out], F32, tag="po")
                # bias via rank-1 (ones^T @ brow) matmul opens the group
                nc.tensor.matmul(po[:], ones[:], bs_t[:], start=True, stop=False)
                for k in range(kin):
                    nc.tensor.matmul(po[:], xT[:, k, :],
                                     ws[:, k * d_out:(k + 1) * d_out],
                                     start=False, stop=(k == kin - 1))
                # two blocks share one output tile -> one 2KB-per-partition
                # store for each pair (halves store descriptor count)
                if j % 2 == 0:
                    opair[0] = wpool.tile([BS, 2, d_out], F16, tag="op",
                                          name="op")
                nc.scalar.activation(opair[0][:, j % 2, :], po[:],
                                     mybir.ActivationFunctionType.Relu)
                if j % 2 == 1:
                    nc.sync.dma_start(
                        yout[:, (j - 1) * d_out:(j + 1) * d_out], opair[0][:])
                elif j == nblk - 1:
                    nc.sync.dma_start(
                        yout[:, j * d_out:(j + 1) * d_out], opair[0][:, 0, :])

            off = 0      # edge-tile offset
            done = -1    # last block whose post-agg chain was issued
            for gi, grp in enumerate(groups):
                tg = sum(t_sched[j] for j in grp)
                g = gpool.tile([128, gmax, d_in], F8, tag="g")
                # the first two gathers are split in two so the early,
                # backlog-free PE gets data at half-generation granularity
                chunks = ([(0, tg // 2), (tg // 2, tg - tg // 2)]
                          if gi <= 1 else [(0, tg)])
                for co, cn in chunks:
                    nc.gpsimd.dma_gather(
                        g[:, co:co + cn, :], htab[:, :],
                        idxts[gi][:, co * 8:(co + cn) * 8],
                        cn * BS, cn * BS, d_in, single_packet=False,
                        queue_num=gi % N_SWDGE_QUEUES,
                    )
                # one S load per gather group (bigger descriptors than
                # per-block loads), on the scalar HWDGE queue
                st = spool.tile([BS, gmax * BS], F8, tag="St")
                nc.scalar.dma_start(st[:, 0:tg * BS],
                                    stab[:, off * BS:(off + tg) * BS])
                if gi == 0:
                    # constants built while block-0's gather drains; kept off
                    # GpSimd's critical path ahead of the first gather
                    make_identity(nc, ident[:])
                    nc.gpsimd.memset(ones[:], 1.0)
                elif gi == 1:
                    # W/bias/norm not needed until the first finish_block
                    # (issued later this iteration); loading them here keeps
                    # the startup ring burst small
                    nc.scalar.dma_start(ws[:], wmat[:])
                    nc.scalar.dma_start(bs_t[:], brow[:])
                    nc.scalar.dma_start(ns_t[:], ndst[:])
                goff = 0  # tile offset within this gather group
                for j in grp:
                    tj = t_sched[j]
                    px = psx.tile([BS, d_in], F32, tag="px")
                    for t in range(tj):
                        pos = goff + t
                        nc.tensor.matmul(px[:], st[:, pos * BS:(pos + 1) * BS],
                                         g[:, goff + t, :], start=(t == 0),
                                         stop=(t == tj - 1))
                    pxs[j] = px
                    goff += tj
                    # issue block j-1's post-agg chain AFTER block j's agg
                    # matmuls so PE isn't blocked waiting on DVE's norm scale
                    if j > 0:
                        finish_block(j - 1)
                    done = j - 1
                off += tg
            for j in range(done + 1, nblk):
                finish_block(j)

    nc.compile()
    return nc


def _assemble(results, meta):
    n_nodes, d_out = meta["n_nodes"], meta["d_out"]
    npc, nblk = meta["npc"], meta["nblk"]
    out = np.empty((n_nodes, d_out), np.float32)
    for c in range(N_CORES):
        res = np.asarray(results[c]["yout"]).astype(np.float32)
        res = res.reshape(BS, nblk, d_out)
        for j in range(nblk):
            b = int(meta["perms"][c][j])
            lo = b * BS
            n = min(BS, npc - lo)
            if n > 0:
                out[c * npc + lo: c * npc + lo + n] = res[:n, j, :]
    return out


def kernel(h, weight, bias, norm, src, dst):
    from concourse.bass_utils import run_bass_kernel_spmd
    nc, in_maps, meta = _prepare(h, weight, bias, norm, src, dst)
    r = run_bass_kernel_spmd(nc, in_maps, list(range(N_CORES)))
    return _assemble(r.results, meta)
